# revision 59
# baseline (speedup 1.0000x reference)
"""Conv2d 3x3 (stride 1, pad 1) Trainium2 Bass kernel — Winograd F(4,3) along H.

Problem: x (32, 128, 56, 56) fp32, kernels (256, 128, 3, 3) fp32, b (256,) fp32
-> out (32, 256, 56, 56) fp32.

Strategy:
  - Data-parallel over batch: 32 images / 8 cores = 4 images per core. SPMD,
    no collectives.
  - Winograd F(4,3) along H (groups of 4 output rows): 6 components per
    group vs 12 direct row-taps, so PE work is half of direct conv
    (~47us/core of matmul columns vs ~94us direct, ~63us for F(2,3)).
    The 3 kw taps stay as ragged accumulating matmuls (col-shifted rhs).
  - The input transform V = B^T d and the weight transform U = G w are
    computed on the HOST (like the bf16 cast): the device receives
    V as [C, 6, 14, 56] bf16 per image (comps in emission order) and
    U as 18 [128cin x 256cout] blocks.
  - Output transform on device: m comps accumulate in PSUM pair-tiles
    (two banks: comp order (m1,m2),(m3,m4),(m0,m5)), ScalarE evicts each
    pair to bf16 SBUF, DVE alone combines (p,q,r,s,t + exact-x2/x4
    scalings) and writes interleaved y rows as bf16 TTs (2x mode).
    GpSimd elementwise measured ~3.2us/op + 1.5us drain here - useless;
    and interleaved-row writes to one tile from two engines serialize
    the queues via the overlap tracker's WAW boxes, so DVE owns them.
      y0 = m0 + (p+r),  y1 = q + 2s,  y2 = p + 4r,  y3 = q + (8s + m5)
      with p/q = m1 +- m2, r/s = m3 +- m4
    Images 0..2 run the epilogue whole-image (fd=784 amortizes the
    ~60ns/op DVE overhead; it hides under the next image's stream);
    the last image runs per-segment (7/4/3 groups) for a short tail.
  - y is stored bf16; the host upcasts to fp32 and adds the bias in the
    same epilogue pass (bias never touches the device).
  - Startup: 11 warm-up matmuls bridge the HAM clock gate; image-0 V
    arrives as per-comp ~100KB chunks and the first weight blocks as a
    64KB transfer (the SDMA rings serialize globally, so the gating
    transfers go first); all input DMA issues ride sync+gpsimd queues
    to keep the scalar queue free for evictions.
"""

import numpy as np
import ml_dtypes

import concourse.bass as bass
import concourse.tile as tile
from concourse import bacc, mybir
from concourse.bass_utils import run_bass_kernel_spmd

N_CORES = 8
N_FULL = 32
N_PER = N_FULL // N_CORES  # 4 images per core
C_IN = 128
C_OUT = 256
H = W = 56
G4 = H // 4         # 14 groups of 4 output rows
FD = 7 * W          # 392 (max segment free dim)

_DT = mybir.dt.bfloat16
_F32 = mybir.dt.float32
_ID = mybir.ActivationFunctionType.Identity
_ADD = mybir.AluOpType.add
_SUB = mybir.AluOpType.subtract
_MULT = mybir.AluOpType.mult

_KW_ORDER = [1, 0, 2]
_KW_POS = {1: 0, 0: 1, 2: 2}
# emission order of Winograd comps: (m1,m2),(m3,m4),(m0,m5) pairs so the
# combine chain overlaps the segment's remaining matmuls
_COMP_ORDER = [1, 2, 3, 4, 0, 5]

SEGS_MID = [(0, 7), (7, 7)]
SEGS_LAST = [(0, 7), (7, 4), (11, 3)]
FDI = G4 * W  # 784: whole-image free dim for the batched epilogue

# F(4,3) transforms, points {0, 1, -1, 2, -2}
_G = np.array(
    [
        [1 / 4, 0, 0],
        [-1 / 6, -1 / 6, -1 / 6],
        [-1 / 6, 1 / 6, -1 / 6],
        [1 / 24, 1 / 12, 1 / 6],
        [1 / 24, -1 / 12, 1 / 6],
        [0, 0, 1],
    ]
)


def _build():
    nc = bacc.Bacc(
        "TRN2",
        target_bir_lowering=False,
        debug=False,
        num_devices=N_CORES,
    )
    vs = nc.dram_tensor(
        "vs", [N_PER, C_IN, 6, G4, W], _DT, kind="ExternalInput"
    ).ap()
    wt = nc.dram_tensor("wt", [C_IN, 18 * C_OUT], _DT, kind="ExternalInput").ap()
    y = nc.dram_tensor("y", [N_PER, C_OUT, H, W], _DT, kind="ExternalOutput").ap()

    with tile.TileContext(nc) as tc:
        with (
            tc.tile_pool(name="const", bufs=1) as const,
            tc.tile_pool(name="vpool", bufs=4) as vpool,
            tc.tile_pool(name="spool", bufs=2) as spool,
            tc.tile_pool(name="qpool", bufs=1) as qpool,
            tc.tile_pool(name="ypool", bufs=3) as ypool,
            tc.tile_pool(name="pspool", bufs=4, space="PSUM") as pspool,
        ):
            # PE warm-up: dummy matmuls during the input-load window keep
            # the HAM activity window busy so the clock gate lifts
            # (1.2 -> 2.4 GHz) right as the real matmuls begin.
            warm = const.tile([128, 512], _DT)
            nc.gpsimd.memset(warm[:], 0.0)
            wps = pspool.tile([128, 2, 512], _F32, tag="ps", name="warm_ps")
            N_WARM = 11
            for i in range(N_WARM):
                nc.tensor.matmul(
                    wps[:, 0, :FD],
                    lhsT=warm[:, :128],
                    rhs=warm[:, :FD],
                    start=(i == 0),
                    stop=(i == N_WARM - 1),
                )

            wt_sb = [
                const.tile([C_IN, 6 * C_OUT], _DT, name=f"wt_sb{i}")
                for i in range(3)
            ]

            def wslice(j, kw, half):
                bi = j * 3 + _KW_POS[kw]
                c0 = (bi % 6) * C_OUT + half * 128
                return wt_sb[bi // 6][:, c0 : c0 + 128]

            # ---- input loads: image 0 paced per-comp for seg 0, the rest
            # whole; weights split over sync + gpsimd queues ----
            # All input DMA issues ride sync+gpsimd queues: the scalar
            # queue must stay free so the first PSUM evictions aren't
            # stuck behind ~0.8us DMA-issue instructions.
            vt = {}
            v0 = vpool.tile([C_IN, 6, G4, W], _DT, tag="v", name="v0")
            nc.sync.dma_start(out=v0[:, 0, 0:7, :], in_=vs[0, :, 0, 0:7, :])
            # comp j0's full kw-triple (blocks 0-2) rides a small transfer
            # so the first accumulation group isn't gated on the full
            # weight tile (block 2 gating kw=2 cost a 1.9us stream gap)
            nc.sync.dma_start(
                out=wt_sb[0][:, : 3 * C_OUT], in_=wt[:, : 3 * C_OUT]
            )
            nc.sync.dma_start(out=v0[:, 1, 0:7, :], in_=vs[0, :, 1, 0:7, :])
            nc.sync.dma_start(
                out=wt_sb[0][:, 3 * C_OUT :], in_=wt[:, 3 * C_OUT : 6 * C_OUT]
            )
            nc.gpsimd.dma_start(
                out=wt_sb[1][:], in_=wt[:, 6 * C_OUT : 12 * C_OUT]
            )
            nc.sync.dma_start(out=wt_sb[2][:], in_=wt[:, 12 * C_OUT :])
            nc.gpsimd.dma_start(
                out=v0[:, 2:6, 0:7, :], in_=vs[0, :, 2:6, 0:7, :]
            )
            nc.sync.dma_start(out=v0[:, 0:2, 7:G4, :], in_=vs[0, :, 0:2, 7:G4, :])
            nc.sync.dma_start(out=v0[:, 2:4, 7:G4, :], in_=vs[0, :, 2:4, 7:G4, :])
            nc.sync.dma_start(out=v0[:, 4:6, 7:G4, :], in_=vs[0, :, 4:6, 7:G4, :])
            vt[0] = v0
            for nn in range(1, N_PER):
                v = vpool.tile([C_IN, 6, G4, W], _DT, tag="v", name=f"v{nn}")
                nc.sync.dma_start(out=v[:], in_=vs[nn])
                vt[nn] = v

            def emit_matmuls(n, v, st, g0, gn, fd0):
                """18 matmuls per half into PSUM pair tiles; evict pairs
                into st at free-dim offset fd0."""
                fdp = gn * W
                for half in range(2):
                    for pj in range(3):  # pair (comps 2pj, 2pj+1)
                        pair = pspool.tile(
                            [128, 2, 512],
                            _F32,
                            tag="ps",
                            name=f"m_{n}_{g0}_{half}_{pj}",
                        )
                        for cj in range(2):
                            j = 2 * pj + cj
                            mi = pair[:, cj, :fdp].rearrange(
                                "p (g w) -> p g w", g=gn
                            )
                            for kw in _KW_ORDER:
                                if kw == 1:
                                    out_ap = mi
                                    rhs = v[:, j, g0 : g0 + gn, :]
                                elif kw == 0:
                                    out_ap = mi[:, :, 1:W]
                                    rhs = v[:, j, g0 : g0 + gn, 0 : W - 1]
                                else:
                                    out_ap = mi[:, :, 0 : W - 1]
                                    rhs = v[:, j, g0 : g0 + gn, 1:W]
                                nc.tensor.matmul(
                                    out_ap,
                                    lhsT=wslice(j, kw, half),
                                    rhs=rhs,
                                    start=(kw == 1),
                                    stop=(kw == 2),
                                )
                        nc.scalar.activation(
                            st[:, half, 2 * pj : 2 * pj + 2, fd0 : fd0 + fdp],
                            pair[:, :, :fdp],
                            _ID,
                        )

            def per_seg_image(n, segs, split_last):
                """Per-segment epilogue entirely on DVE,
                for the last image: short post-stream tail."""
                v = vt[n]
                for si, (g0, gn) in enumerate(segs):
                    fdp = gn * W
                    last = split_last and si == len(segs) - 1
                    st = spool.tile(
                        [128, 2, 6, FD], _DT, tag="st", name=f"stl_{n}_{si}"
                    )
                    emit_matmuls(n, v, st, g0, gn, 0)

                    def evl(slot, st=st, fdp=fdp):
                        return st[:, :, slot, :fdp]

                    def vall(name_, n=n, si=si, fdp=fdp):
                        tl = qpool.tile(
                            [128, 2, FD],
                            _DT,
                            tag=f"{name_}l",
                            name=f"{name_}l_{n}_{si}",
                        )
                        return tl[:, :, :fdp]

                    p = vall("p")
                    q = vall("q")
                    r = vall("r")
                    s = vall("s")
                    t = vall("t")
                    s2 = vall("s2")
                    r4 = vall("r4")
                    s8 = vall("s8")
                    u5 = vall("u5")
                    nc.vector.tensor_add(p, evl(0), evl(1))
                    nc.vector.tensor_sub(q, evl(0), evl(1))
                    nc.vector.tensor_add(r, evl(2), evl(3))
                    nc.vector.tensor_sub(s, evl(2), evl(3))
                    nc.vector.tensor_add(t, p, r)
                    nc.vector.tensor_scalar_mul(s2, s, 2.0)
                    nc.vector.tensor_scalar_mul(r4, r, 4.0)
                    nc.vector.tensor_scalar_mul(s8, s2, 4.0)
                    nc.vector.tensor_add(u5, s8, evl(5))

                    for half in range(2):
                        def as3(ap, gn=gn):
                            return ap.rearrange("p (g w) -> p g w", g=gn)

                        yt = ypool.tile(
                            [128, 28, W],
                            _DT,
                            tag="yt",
                            name=f"yl_{n}_{si}_{half}",
                        )
                        rows = 4 * gn
                        nc.vector.tensor_add(
                            yt[:, 0:rows:4, :],
                            as3(evl(4)[:, half]),
                            as3(t[:, half]),
                        )
                        nc.vector.tensor_add(
                            yt[:, 1:rows:4, :], as3(q[:, half]), as3(s2[:, half])
                        )
                        nc.vector.tensor_add(
                            yt[:, 2:rows:4, :], as3(p[:, half]), as3(r4[:, half])
                        )
                        nc.vector.tensor_add(
                            yt[:, 3:rows:4, :], as3(q[:, half]), as3(u5[:, half])
                        )

                        r0 = 4 * g0
                        y_slice = y[
                            n, half * 128 : (half + 1) * 128, r0 : r0 + rows, :
                        ]
                        if last and half == 1:
                            # split the final store so its DMA drain
                            # doesn't gate the end barrier on one queue
                            nc.sync.dma_start(
                                out=y_slice[:, 0 : 2 * gn, :],
                                in_=yt[:, 0 : 2 * gn, :],
                            )
                            nc.scalar.dma_start(
                                out=y_slice[:, 2 * gn : rows, :],
                                in_=yt[:, 2 * gn : rows, :],
                            )
                        else:
                            nc.sync.dma_start(out=y_slice, in_=yt[:, :rows, :])

            per_seg_image(0, SEGS_MID, False)

            # ---- images 1..2: whole-image epilogue (fd=784 ops amortize
            # the DVE per-op overhead; it hides under the NEXT image's
            # matmul stream).
            for n in range(1, N_PER - 1):
                v = vt[n]
                st = spool.tile(
                    [128, 2, 6, FDI], _DT, tag="sti", name=f"st_{n}"
                )
                for g0, gn in SEGS_MID:
                    emit_matmuls(n, v, st, g0, gn, g0 * W)

                # st slots (emission order): 0=m1 1=m2 2=m3 3=m4 4=m0 5=m5
                def ev(slot, st=st):
                    return st[:, :, slot, :]

                def val(name_, n=n):
                    return qpool.tile(
                        [128, 2, FDI], _DT, tag=name_, name=f"{name_}_{n}"
                    )

                p = val("p")
                q = val("q")
                r = val("r")
                s = val("s")
                t = val("t")
                s2 = val("s2")
                r4 = val("r4")
                s8 = val("s8")
                u5 = val("u5")
                nc.vector.tensor_add(p, ev(0), ev(1))
                nc.vector.tensor_sub(q, ev(0), ev(1))
                nc.vector.tensor_add(r, ev(2), ev(3))
                nc.vector.tensor_sub(s, ev(2), ev(3))
                nc.vector.tensor_add(t, p, r)
                nc.vector.tensor_scalar_mul(s2, s, 2.0)
                nc.vector.tensor_scalar_mul(r4, r, 4.0)
                nc.vector.tensor_scalar_mul(s8, s2, 4.0)
                nc.vector.tensor_add(u5, s8, ev(5))

                for half in range(2):
                    def as3(ap):
                        return ap.rearrange("p (g w) -> p g w", g=G4)

                    yt = ypool.tile(
                        [128, H, W], _DT, tag="yti", name=f"y_{n}_{half}"
                    )
                    nc.vector.tensor_add(
                        yt[:, 0:H:4, :], as3(ev(4)[:, half]), as3(t[:, half])
                    )
                    nc.vector.tensor_add(
                        yt[:, 1:H:4, :], as3(q[:, half]), as3(s2[:, half])
                    )
                    nc.vector.tensor_add(
                        yt[:, 2:H:4, :], as3(p[:, half]), as3(r4[:, half])
                    )
                    nc.vector.tensor_add(
                        yt[:, 3:H:4, :], as3(q[:, half]), as3(u5[:, half])
                    )
                    nc.sync.dma_start(
                        out=y[n, half * 128 : (half + 1) * 128, :, :], in_=yt[:]
                    )

            per_seg_image(N_PER - 1, SEGS_LAST, True)
    nc.compile()
    return nc


_NC = None


def _get_nc():
    global _NC
    if _NC is None:
        _NC = _build()
    return _NC


def _prep_inputs(x, kernels, b):
    bf16 = ml_dtypes.bfloat16
    x = np.asarray(x, dtype=np.float32)
    w = np.asarray(kernels, dtype=np.float32)  # [O, C, kh, kw]

    # host input transform V = B^T d along H (comps in emission order)
    xp = np.pad(x, ((0, 0), (0, 0), (1, 1), (0, 0)))  # rows -1..56
    # d[r] for group g is xp[:, :, 4g+r, :]
    d = [xp[:, :, r : r + 4 * G4 : 4, :] for r in range(6)]  # each [N,C,14,W]
    a = d[0] - d[2]
    i_ = d[4] - d[2]
    c = d[1] + d[2]
    e = d[3] + d[4]
    f = d[1] - d[2]
    gg = d[3] - d[4]
    h = d[3] - d[1]
    k = d[3] - d[5]
    Vfull = [4 * a + i_, -4 * c + e, 4 * f - gg, 2 * h + i_, -2 * h + i_, -4 * h - k]
    V = np.stack([Vfull[ci] for ci in _COMP_ORDER], axis=2)  # [N,C,6,14,W]
    vsb = np.ascontiguousarray(V).astype(bf16)

    # host weight transform U[i] = sum_r G[i,r] w[:,:,r,:], blocks in
    # (emission comp, kw 1/0/2) order, each [C, O]
    U = np.einsum("ir,ocrk->iock", _G, w)  # [6, O, C, kw]
    blocks = []
    for j in _COMP_ORDER:
        for kw in _KW_ORDER:
            blocks.append(U[j, :, :, kw].T)
    wtb = np.ascontiguousarray(np.concatenate(blocks, axis=1)).astype(bf16)
    return vsb, wtb


def kernel(x, kernels, b):
    nc = _get_nc()
    vsb, wtb = _prep_inputs(x, kernels, b)
    in_maps = [
        {"vs": vsb[i * N_PER : (i + 1) * N_PER], "wt": wtb}
        for i in range(N_CORES)
    ]
    res = run_bass_kernel_spmd(nc, in_maps, core_ids=list(range(N_CORES)))
    bias = np.asarray(b, dtype=np.float32).reshape(1, C_OUT, 1, 1)
    out = np.concatenate(
        [
            r["y"].astype(np.float32).reshape(N_PER, C_OUT, H, W)
            for r in res.results
        ],
        axis=0,
    )
    out += bias
    return np.ascontiguousarray(out, dtype=np.float32)


# revision 60
# speedup vs baseline: 1.0190x; 1.0190x over previous
"""Conv2d 3x3 (stride 1, pad 1) Trainium2 Bass kernel — Winograd F(4,3) along H.

Problem: x (32, 128, 56, 56) fp32, kernels (256, 128, 3, 3) fp32, b (256,) fp32
-> out (32, 256, 56, 56) fp32.

Strategy:
  - Data-parallel over batch: 32 images / 8 cores = 4 images per core. SPMD,
    no collectives.
  - Winograd F(4,3) along H (groups of 4 output rows): 6 components per
    group vs 12 direct row-taps, so PE work is half of direct conv
    (~47us/core of matmul columns vs ~94us direct, ~63us for F(2,3)).
    The 3 kw taps stay as ragged accumulating matmuls (col-shifted rhs).
  - The input transform V = B^T d and the weight transform U = G w are
    computed on the HOST (like the bf16 cast): the device receives
    V as [C, 6, 14, 56] bf16 per image (comps in emission order) and
    U as 18 [128cin x 256cout] blocks.
  - Output transform on device: m comps accumulate in PSUM pair-tiles
    (two banks: comp order (m1,m2),(m3,m4),(m0,m5)), ScalarE evicts each
    pair to bf16 SBUF, DVE alone combines (p,q,r,s,t + exact-x2/x4
    scalings) and writes interleaved y rows as bf16 TTs (2x mode).
    GpSimd elementwise measured ~3.2us/op + 1.5us drain here - useless;
    and interleaved-row writes to one tile from two engines serialize
    the queues via the overlap tracker's WAW boxes, so DVE owns them.
      y0 = m0 + (p+r),  y1 = q + 2s,  y2 = p + 4r,  y3 = q + (8s + m5)
      with p/q = m1 +- m2, r/s = m3 +- m4
    Images 0..2 run the epilogue whole-image (fd=784 amortizes the
    ~60ns/op DVE overhead; it hides under the next image's stream);
    the last image runs per-segment (7/4/3 groups) for a short tail.
  - y is stored bf16; the host upcasts to fp32 and adds the bias in the
    same epilogue pass (bias never touches the device).
  - Startup: 11 warm-up matmuls bridge the HAM clock gate; image-0 V
    arrives as per-comp ~100KB chunks and the first weight blocks as a
    64KB transfer (the SDMA rings serialize globally, so the gating
    transfers go first); all input DMA issues ride sync+gpsimd queues
    to keep the scalar queue free for evictions.
"""

import numpy as np
import ml_dtypes

import concourse.bass as bass
import concourse.tile as tile
from concourse import bacc, mybir
from concourse.bass_utils import run_bass_kernel_spmd

N_CORES = 8
N_FULL = 32
N_PER = N_FULL // N_CORES  # 4 images per core
C_IN = 128
C_OUT = 256
H = W = 56
G4 = H // 4         # 14 groups of 4 output rows
FD = 7 * W          # 392 (max segment free dim)

_DT = mybir.dt.bfloat16
_F32 = mybir.dt.float32
_ID = mybir.ActivationFunctionType.Identity
_ADD = mybir.AluOpType.add
_SUB = mybir.AluOpType.subtract
_MULT = mybir.AluOpType.mult

_KW_ORDER = [1, 0, 2]
_KW_POS = {1: 0, 0: 1, 2: 2}
# emission order of Winograd comps: (m1,m2),(m3,m4),(m0,m5) pairs so the
# combine chain overlaps the segment's remaining matmuls
_COMP_ORDER = [1, 2, 3, 4, 0, 5]

SEGS_MID = [(0, 7), (7, 7)]
SEGS_LAST = [(0, 7), (7, 4), (11, 3)]
FDI = G4 * W  # 784: whole-image free dim for the batched epilogue

# F(4,3) transforms, points {0, 1, -1, 2, -2}
_G = np.array(
    [
        [1 / 4, 0, 0],
        [-1 / 6, -1 / 6, -1 / 6],
        [-1 / 6, 1 / 6, -1 / 6],
        [1 / 24, 1 / 12, 1 / 6],
        [1 / 24, -1 / 12, 1 / 6],
        [0, 0, 1],
    ]
)


def _build():
    nc = bacc.Bacc(
        "TRN2",
        target_bir_lowering=False,
        debug=False,
        num_devices=N_CORES,
    )
    vs = nc.dram_tensor(
        "vs", [N_PER, C_IN, 6, G4, W], _DT, kind="ExternalInput"
    ).ap()
    wt = nc.dram_tensor("wt", [C_IN, 18 * C_OUT], _DT, kind="ExternalInput").ap()
    y = nc.dram_tensor("y", [N_PER, C_OUT, H, W], _DT, kind="ExternalOutput").ap()

    with tile.TileContext(nc) as tc:
        with (
            tc.tile_pool(name="const", bufs=1) as const,
            tc.tile_pool(name="vpool", bufs=4) as vpool,
            tc.tile_pool(name="spool", bufs=2) as spool,
            tc.tile_pool(name="qpool", bufs=1) as qpool,
            tc.tile_pool(name="ypool", bufs=3) as ypool,
            tc.tile_pool(name="pspool", bufs=4, space="PSUM") as pspool,
        ):
            # PE warm-up: dummy matmuls during the input-load window keep
            # the HAM activity window busy so the clock gate lifts
            # (1.2 -> 2.4 GHz) right as the real matmuls begin.
            warm = const.tile([128, 512], _DT)
            nc.gpsimd.memset(warm[:], 0.0)
            wps = pspool.tile([128, 2, 512], _F32, tag="ps", name="warm_ps")
            N_WARM = 11
            for i in range(N_WARM):
                nc.tensor.matmul(
                    wps[:, 0, :FD],
                    lhsT=warm[:, :128],
                    rhs=warm[:, :FD],
                    start=(i == 0),
                    stop=(i == N_WARM - 1),
                )

            wt_sb = [
                const.tile([C_IN, 6 * C_OUT], _DT, name=f"wt_sb{i}")
                for i in range(3)
            ]

            def wslice(j, kw, half):
                bi = j * 3 + _KW_POS[kw]
                c0 = (bi % 6) * C_OUT + half * 128
                return wt_sb[bi // 6][:, c0 : c0 + 128]

            # ---- input loads: image 0 paced per-comp for seg 0, the rest
            # whole; weights split over sync + gpsimd queues ----
            # All input DMA issues ride sync+gpsimd queues: the scalar
            # queue must stay free so the first PSUM evictions aren't
            # stuck behind ~0.8us DMA-issue instructions.
            vt = {}
            v0 = vpool.tile([C_IN, 6, G4, W], _DT, tag="v", name="v0")
            nc.sync.dma_start(out=v0[:, 0, 0:7, :], in_=vs[0, :, 0, 0:7, :])
            # comp j0's full kw-triple (blocks 0-2) rides a small transfer
            # so the first accumulation group isn't gated on the full
            # weight tile (block 2 gating kw=2 cost a 1.9us stream gap)
            nc.sync.dma_start(
                out=wt_sb[0][:, : 3 * C_OUT], in_=wt[:, : 3 * C_OUT]
            )
            nc.sync.dma_start(out=v0[:, 1, 0:7, :], in_=vs[0, :, 1, 0:7, :])
            nc.sync.dma_start(
                out=wt_sb[0][:, 3 * C_OUT :], in_=wt[:, 3 * C_OUT : 6 * C_OUT]
            )
            nc.gpsimd.dma_start(
                out=wt_sb[1][:], in_=wt[:, 6 * C_OUT : 12 * C_OUT]
            )
            nc.sync.dma_start(out=wt_sb[2][:], in_=wt[:, 12 * C_OUT :])
            nc.gpsimd.dma_start(
                out=v0[:, 2:6, 0:7, :], in_=vs[0, :, 2:6, 0:7, :]
            )
            nc.sync.dma_start(out=v0[:, 0:2, 7:G4, :], in_=vs[0, :, 0:2, 7:G4, :])
            nc.sync.dma_start(out=v0[:, 2:4, 7:G4, :], in_=vs[0, :, 2:4, 7:G4, :])
            nc.sync.dma_start(out=v0[:, 4:6, 7:G4, :], in_=vs[0, :, 4:6, 7:G4, :])
            vt[0] = v0
            for nn in range(1, N_PER):
                v = vpool.tile([C_IN, 6, G4, W], _DT, tag="v", name=f"v{nn}")
                nc.sync.dma_start(out=v[:], in_=vs[nn])
                vt[nn] = v

            def emit_matmuls(n, v, st, g0, gn, fd0):
                """18 matmuls per half into PSUM pair tiles; evict pairs
                into st at free-dim offset fd0."""
                fdp = gn * W
                for half in range(2):
                    for pj in range(3):  # pair (comps 2pj, 2pj+1)
                        pair = pspool.tile(
                            [128, 2, 512],
                            _F32,
                            tag="ps",
                            name=f"m_{n}_{g0}_{half}_{pj}",
                        )
                        for cj in range(2):
                            j = 2 * pj + cj
                            mi = pair[:, cj, :fdp].rearrange(
                                "p (g w) -> p g w", g=gn
                            )
                            for kw in _KW_ORDER:
                                if kw == 1:
                                    out_ap = mi
                                    rhs = v[:, j, g0 : g0 + gn, :]
                                elif kw == 0:
                                    out_ap = mi[:, :, 1:W]
                                    rhs = v[:, j, g0 : g0 + gn, 0 : W - 1]
                                else:
                                    out_ap = mi[:, :, 0 : W - 1]
                                    rhs = v[:, j, g0 : g0 + gn, 1:W]
                                nc.tensor.matmul(
                                    out_ap,
                                    lhsT=wslice(j, kw, half),
                                    rhs=rhs,
                                    start=(kw == 1),
                                    stop=(kw == 2),
                                )
                        nc.scalar.activation(
                            st[:, half, 2 * pj : 2 * pj + 2, fd0 : fd0 + fdp],
                            pair[:, :, :fdp],
                            _ID,
                        )

            def per_seg_image(n, segs, split_last, si0=0):
                """Per-segment epilogue entirely on DVE,
                for the last image: short post-stream tail."""
                v = vt[n]
                for si, (g0, gn) in enumerate(segs, start=si0):
                    fdp = gn * W
                    last = split_last and si == len(segs) - 1
                    st = spool.tile(
                        [128, 2, 6, FD], _DT, tag="st", name=f"stl_{n}_{si}"
                    )
                    emit_matmuls(n, v, st, g0, gn, 0)

                    def evl(slot, st=st, fdp=fdp):
                        return st[:, :, slot, :fdp]

                    def vall(name_, n=n, si=si, fdp=fdp):
                        tl = qpool.tile(
                            [128, 2, FD],
                            _DT,
                            tag=f"{name_}l",
                            name=f"{name_}l_{n}_{si}",
                        )
                        return tl[:, :, :fdp]

                    p = vall("p")
                    q = vall("q")
                    r = vall("r")
                    s = vall("s")
                    t = vall("t")
                    s2 = vall("s2")
                    r4 = vall("r4")
                    s8 = vall("s8")
                    u5 = vall("u5")
                    nc.vector.tensor_add(p, evl(0), evl(1))
                    nc.vector.tensor_sub(q, evl(0), evl(1))
                    nc.vector.tensor_add(r, evl(2), evl(3))
                    nc.vector.tensor_sub(s, evl(2), evl(3))
                    nc.vector.tensor_add(t, p, r)
                    nc.vector.tensor_scalar_mul(s2, s, 2.0)
                    nc.vector.tensor_scalar_mul(r4, r, 4.0)
                    nc.vector.tensor_scalar_mul(s8, s2, 4.0)
                    nc.vector.tensor_add(u5, s8, evl(5))

                    for half in range(2):
                        def as3(ap, gn=gn):
                            return ap.rearrange("p (g w) -> p g w", g=gn)

                        yt = ypool.tile(
                            [128, 28, W],
                            _DT,
                            tag="yt",
                            name=f"yl_{n}_{si}_{half}",
                        )
                        rows = 4 * gn
                        nc.vector.tensor_add(
                            yt[:, 0:rows:4, :],
                            as3(evl(4)[:, half]),
                            as3(t[:, half]),
                        )
                        nc.vector.tensor_add(
                            yt[:, 1:rows:4, :], as3(q[:, half]), as3(s2[:, half])
                        )
                        nc.vector.tensor_add(
                            yt[:, 2:rows:4, :], as3(p[:, half]), as3(r4[:, half])
                        )
                        nc.vector.tensor_add(
                            yt[:, 3:rows:4, :], as3(q[:, half]), as3(u5[:, half])
                        )

                        r0 = 4 * g0
                        y_slice = y[
                            n, half * 128 : (half + 1) * 128, r0 : r0 + rows, :
                        ]
                        if last and half == 1:
                            # split the final store so its DMA drain
                            # doesn't gate the end barrier on one queue
                            nc.sync.dma_start(
                                out=y_slice[:, 0 : 2 * gn, :],
                                in_=yt[:, 0 : 2 * gn, :],
                            )
                            nc.scalar.dma_start(
                                out=y_slice[:, 2 * gn : rows, :],
                                in_=yt[:, 2 * gn : rows, :],
                            )
                        else:
                            nc.sync.dma_start(out=y_slice, in_=yt[:, :rows, :])

            per_seg_image(0, SEGS_MID, False)

            # ---- images 1..2: whole-image epilogue (fd=784 ops amortize
            # the DVE per-op overhead; it hides under the NEXT image's
            # matmul stream).
            for n in range(1, N_PER - 1):
                v = vt[n]
                st = spool.tile(
                    [128, 2, 6, FDI], _DT, tag="sti", name=f"st_{n}"
                )
                for g0, gn in SEGS_MID:
                    emit_matmuls(n, v, st, g0, gn, g0 * W)

                # st slots (emission order): 0=m1 1=m2 2=m3 3=m4 4=m0 5=m5
                def ev(slot, st=st):
                    return st[:, :, slot, :]

                def val(name_, n=n):
                    return qpool.tile(
                        [128, 2, FDI], _DT, tag=name_, name=f"{name_}_{n}"
                    )

                p = val("p")
                q = val("q")
                r = val("r")
                s = val("s")
                t = val("t")
                s2 = val("s2")
                r4 = val("r4")
                s8 = val("s8")
                u5 = val("u5")
                nc.vector.tensor_add(p, ev(0), ev(1))
                nc.vector.tensor_sub(q, ev(0), ev(1))
                nc.vector.tensor_add(r, ev(2), ev(3))
                nc.vector.tensor_sub(s, ev(2), ev(3))
                nc.vector.tensor_add(t, p, r)
                nc.vector.tensor_scalar_mul(s2, s, 2.0)
                nc.vector.tensor_scalar_mul(r4, r, 4.0)
                nc.vector.tensor_scalar_mul(s8, s2, 4.0)
                nc.vector.tensor_add(u5, s8, ev(5))

                def emit_y(n=n, ev=ev, p=p, q=q, t=t, s2=s2, r4=r4, u5=u5):
                    for half in range(2):
                        def as3(ap):
                            return ap.rearrange("p (g w) -> p g w", g=G4)

                        yt = ypool.tile(
                            [128, H, W], _DT, tag="yti", name=f"y_{n}_{half}"
                        )
                        nc.vector.tensor_add(
                            yt[:, 0:H:4, :], as3(ev(4)[:, half]), as3(t[:, half])
                        )
                        nc.vector.tensor_add(
                            yt[:, 1:H:4, :], as3(q[:, half]), as3(s2[:, half])
                        )
                        nc.vector.tensor_add(
                            yt[:, 2:H:4, :], as3(p[:, half]), as3(r4[:, half])
                        )
                        nc.vector.tensor_add(
                            yt[:, 3:H:4, :], as3(q[:, half]), as3(u5[:, half])
                        )
                        nc.sync.dma_start(
                            out=y[n, half * 128 : (half + 1) * 128, :, :],
                            in_=yt[:],
                        )

                if n < N_PER - 2:
                    emit_y()
                else:
                    # defer image 2's y-writes (store-feeding only, no
                    # downstream deps) until after image 3's seg-0 vals:
                    # this releases seg-0's st tile ~4us earlier, unblocking
                    # the eviction -> PSUM-pair chain that stalls image 3's
                    # later matmuls
                    deferred_y = emit_y

            per_seg_image(N_PER - 1, SEGS_LAST[:1], False)
            deferred_y()
            per_seg_image(N_PER - 1, SEGS_LAST[1:], True, si0=1)
    nc.compile()
    return nc


_NC = None


def _get_nc():
    global _NC
    if _NC is None:
        _NC = _build()
    return _NC


def _prep_inputs(x, kernels, b):
    bf16 = ml_dtypes.bfloat16
    x = np.asarray(x, dtype=np.float32)
    w = np.asarray(kernels, dtype=np.float32)  # [O, C, kh, kw]

    # host input transform V = B^T d along H (comps in emission order)
    xp = np.pad(x, ((0, 0), (0, 0), (1, 1), (0, 0)))  # rows -1..56
    # d[r] for group g is xp[:, :, 4g+r, :]
    d = [xp[:, :, r : r + 4 * G4 : 4, :] for r in range(6)]  # each [N,C,14,W]
    a = d[0] - d[2]
    i_ = d[4] - d[2]
    c = d[1] + d[2]
    e = d[3] + d[4]
    f = d[1] - d[2]
    gg = d[3] - d[4]
    h = d[3] - d[1]
    k = d[3] - d[5]
    Vfull = [4 * a + i_, -4 * c + e, 4 * f - gg, 2 * h + i_, -2 * h + i_, -4 * h - k]
    V = np.stack([Vfull[ci] for ci in _COMP_ORDER], axis=2)  # [N,C,6,14,W]
    vsb = np.ascontiguousarray(V).astype(bf16)

    # host weight transform U[i] = sum_r G[i,r] w[:,:,r,:], blocks in
    # (emission comp, kw 1/0/2) order, each [C, O]
    U = np.einsum("ir,ocrk->iock", _G, w)  # [6, O, C, kw]
    blocks = []
    for j in _COMP_ORDER:
        for kw in _KW_ORDER:
            blocks.append(U[j, :, :, kw].T)
    wtb = np.ascontiguousarray(np.concatenate(blocks, axis=1)).astype(bf16)
    return vsb, wtb


def kernel(x, kernels, b):
    nc = _get_nc()
    vsb, wtb = _prep_inputs(x, kernels, b)
    in_maps = [
        {"vs": vsb[i * N_PER : (i + 1) * N_PER], "wt": wtb}
        for i in range(N_CORES)
    ]
    res = run_bass_kernel_spmd(nc, in_maps, core_ids=list(range(N_CORES)))
    bias = np.asarray(b, dtype=np.float32).reshape(1, C_OUT, 1, 1)
    out = np.concatenate(
        [
            r["y"].astype(np.float32).reshape(N_PER, C_OUT, H, W)
            for r in res.results
        ],
        axis=0,
    )
    out += bias
    return np.ascontiguousarray(out, dtype=np.float32)


# revision 61
# speedup vs baseline: 1.0298x; 1.0106x over previous
"""Conv2d 3x3 (stride 1, pad 1) Trainium2 Bass kernel — Winograd F(4,3) along H.

Problem: x (32, 128, 56, 56) fp32, kernels (256, 128, 3, 3) fp32, b (256,) fp32
-> out (32, 256, 56, 56) fp32.

Strategy:
  - Data-parallel over batch: 32 images / 8 cores = 4 images per core. SPMD,
    no collectives.
  - Winograd F(4,3) along H (groups of 4 output rows): 6 components per
    group vs 12 direct row-taps, so PE work is half of direct conv
    (~47us/core of matmul columns vs ~94us direct, ~63us for F(2,3)).
    The 3 kw taps stay as ragged accumulating matmuls (col-shifted rhs).
  - The input transform V = B^T d and the weight transform U = G w are
    computed on the HOST (like the bf16 cast): the device receives
    V as [C, 6, 14, 56] bf16 per image (comps in emission order) and
    U as 18 [128cin x 256cout] blocks.
  - Output transform on device: m comps accumulate in PSUM pair-tiles
    (two banks: comp order (m1,m2),(m3,m4),(m0,m5)), ScalarE evicts each
    pair to bf16 SBUF, DVE alone combines (p,q,r,s,t + exact-x2/x4
    scalings) and writes interleaved y rows as bf16 TTs (2x mode).
    GpSimd elementwise measured ~3.2us/op + 1.5us drain here - useless;
    and interleaved-row writes to one tile from two engines serialize
    the queues via the overlap tracker's WAW boxes, so DVE owns them.
      y0 = m0 + (p+r),  y1 = q + 2s,  y2 = p + 4r,  y3 = q + (8s + m5)
      with p/q = m1 +- m2, r/s = m3 +- m4
    Images 0..2 run the epilogue whole-image (fd=784 amortizes the
    ~60ns/op DVE overhead; it hides under the next image's stream);
    the last image runs per-segment (7/4/3 groups) for a short tail.
  - y is stored bf16; the host upcasts to fp32 and adds the bias in the
    same epilogue pass (bias never touches the device).
  - Startup: 11 warm-up matmuls bridge the HAM clock gate; image-0 V
    arrives as per-comp ~100KB chunks and the first weight blocks as a
    64KB transfer (the SDMA rings serialize globally, so the gating
    transfers go first); all input DMA issues ride sync+gpsimd queues
    to keep the scalar queue free for evictions.
"""

import numpy as np
import ml_dtypes

import concourse.bass as bass
import concourse.tile as tile
from concourse import bacc, mybir
from concourse.bass_utils import run_bass_kernel_spmd

N_CORES = 8
N_FULL = 32
N_PER = N_FULL // N_CORES  # 4 images per core
C_IN = 128
C_OUT = 256
H = W = 56
G4 = H // 4         # 14 groups of 4 output rows
FD = 7 * W          # 392 (max segment free dim)

_DT = mybir.dt.bfloat16
_F32 = mybir.dt.float32
_ID = mybir.ActivationFunctionType.Identity
_ADD = mybir.AluOpType.add
_SUB = mybir.AluOpType.subtract
_MULT = mybir.AluOpType.mult

_KW_ORDER = [1, 0, 2]
_KW_POS = {1: 0, 0: 1, 2: 2}
# emission order of Winograd comps: (m1,m2),(m3,m4),(m0,m5) pairs so the
# combine chain overlaps the segment's remaining matmuls
_COMP_ORDER = [1, 2, 3, 4, 0, 5]

SEGS_MID = [(0, 7), (7, 7)]
SEGS_LAST = [(0, 7), (7, 4), (11, 3)]
FDI = G4 * W  # 784: whole-image free dim for the batched epilogue

# F(4,3) transforms, points {0, 1, -1, 2, -2}
_G = np.array(
    [
        [1 / 4, 0, 0],
        [-1 / 6, -1 / 6, -1 / 6],
        [-1 / 6, 1 / 6, -1 / 6],
        [1 / 24, 1 / 12, 1 / 6],
        [1 / 24, -1 / 12, 1 / 6],
        [0, 0, 1],
    ]
)


def _build():
    nc = bacc.Bacc(
        "TRN2",
        target_bir_lowering=False,
        debug=False,
        num_devices=N_CORES,
    )
    vs = nc.dram_tensor(
        "vs", [N_PER, C_IN, 6, G4, W], _DT, kind="ExternalInput"
    ).ap()
    wt = nc.dram_tensor("wt", [C_IN, 18 * C_OUT], _DT, kind="ExternalInput").ap()
    y = nc.dram_tensor("y", [N_PER, C_OUT, H, W], _DT, kind="ExternalOutput").ap()

    with tile.TileContext(nc) as tc:
        with (
            tc.tile_pool(name="const", bufs=1) as const,
            tc.tile_pool(name="vpool", bufs=4) as vpool,
            tc.tile_pool(name="spool", bufs=2) as spool,
            tc.tile_pool(name="qpool", bufs=1) as qpool,
            tc.tile_pool(name="ypool", bufs=3) as ypool,
            tc.tile_pool(name="pspool", bufs=4, space="PSUM") as pspool,
        ):
            # PE warm-up: dummy matmuls during the input-load window keep
            # the HAM activity window busy so the clock gate lifts
            # (1.2 -> 2.4 GHz) right as the real matmuls begin.
            warm = const.tile([128, 512], _DT)
            nc.gpsimd.memset(warm[:], 0.0)
            wps = pspool.tile([128, 2, 512], _F32, tag="ps", name="warm_ps")
            N_WARM = 11
            for i in range(N_WARM):
                nc.tensor.matmul(
                    wps[:, 0, :FD],
                    lhsT=warm[:, :128],
                    rhs=warm[:, :FD],
                    start=(i == 0),
                    stop=(i == N_WARM - 1),
                )

            wt_sb = [
                const.tile([C_IN, 6 * C_OUT], _DT, name=f"wt_sb{i}")
                for i in range(3)
            ]

            def wslice(j, kw, half):
                bi = j * 3 + _KW_POS[kw]
                c0 = (bi % 6) * C_OUT + half * 128
                return wt_sb[bi // 6][:, c0 : c0 + 128]

            # ---- input loads: image 0 paced per-comp for seg 0, the rest
            # whole; weights split over sync + gpsimd queues ----
            # All input DMA issues ride sync+gpsimd queues: the scalar
            # queue must stay free so the first PSUM evictions aren't
            # stuck behind ~0.8us DMA-issue instructions.
            vt = {}
            v0 = vpool.tile([C_IN, 6, G4, W], _DT, tag="v", name="v0")
            nc.sync.dma_start(out=v0[:, 0, 0:7, :], in_=vs[0, :, 0, 0:7, :])
            # comp j0's full kw-triple (blocks 0-2) rides a small transfer
            # so the first accumulation group isn't gated on the full
            # weight tile (block 2 gating kw=2 cost a 1.9us stream gap)
            nc.sync.dma_start(
                out=wt_sb[0][:, : 3 * C_OUT], in_=wt[:, : 3 * C_OUT]
            )
            nc.sync.dma_start(out=v0[:, 1, 0:7, :], in_=vs[0, :, 1, 0:7, :])
            nc.sync.dma_start(
                out=wt_sb[0][:, 3 * C_OUT :], in_=wt[:, 3 * C_OUT : 6 * C_OUT]
            )
            nc.gpsimd.dma_start(
                out=wt_sb[1][:], in_=wt[:, 6 * C_OUT : 12 * C_OUT]
            )
            nc.sync.dma_start(out=wt_sb[2][:], in_=wt[:, 12 * C_OUT :])
            nc.gpsimd.dma_start(
                out=v0[:, 2:6, 0:7, :], in_=vs[0, :, 2:6, 0:7, :]
            )
            nc.sync.dma_start(out=v0[:, 0:2, 7:G4, :], in_=vs[0, :, 0:2, 7:G4, :])
            nc.sync.dma_start(out=v0[:, 2:4, 7:G4, :], in_=vs[0, :, 2:4, 7:G4, :])
            nc.sync.dma_start(out=v0[:, 4:6, 7:G4, :], in_=vs[0, :, 4:6, 7:G4, :])
            vt[0] = v0
            for nn in range(1, N_PER):
                v = vpool.tile([C_IN, 6, G4, W], _DT, tag="v", name=f"v{nn}")
                nc.sync.dma_start(out=v[:], in_=vs[nn])
                vt[nn] = v

            def emit_matmuls(n, v, st, g0, gn, fd0):
                """18 matmuls per half into PSUM pair tiles; evict pairs
                into st at free-dim offset fd0."""
                fdp = gn * W
                for half in range(2):
                    for pj in range(3):  # pair (comps 2pj, 2pj+1)
                        pair = pspool.tile(
                            [128, 2, 512],
                            _F32,
                            tag="ps",
                            name=f"m_{n}_{g0}_{half}_{pj}",
                        )
                        for cj in range(2):
                            j = 2 * pj + cj
                            mi = pair[:, cj, :fdp].rearrange(
                                "p (g w) -> p g w", g=gn
                            )
                            for kw in _KW_ORDER:
                                if kw == 1:
                                    out_ap = mi
                                    rhs = v[:, j, g0 : g0 + gn, :]
                                elif kw == 0:
                                    out_ap = mi[:, :, 1:W]
                                    rhs = v[:, j, g0 : g0 + gn, 0 : W - 1]
                                else:
                                    out_ap = mi[:, :, 0 : W - 1]
                                    rhs = v[:, j, g0 : g0 + gn, 1:W]
                                nc.tensor.matmul(
                                    out_ap,
                                    lhsT=wslice(j, kw, half),
                                    rhs=rhs,
                                    start=(kw == 1),
                                    stop=(kw == 2),
                                )
                        nc.scalar.activation(
                            st[:, half, 2 * pj : 2 * pj + 2, fd0 : fd0 + fdp],
                            pair[:, :, :fdp],
                            _ID,
                        )

            def per_seg_image(n, segs, split_last, si0=0):
                """Per-segment epilogue entirely on DVE,
                for the last image: short post-stream tail."""
                v = vt[n]
                for si, (g0, gn) in enumerate(segs, start=si0):
                    fdp = gn * W
                    last = split_last and si == len(segs) - 1
                    st = spool.tile(
                        [128, 2, 6, FD], _DT, tag="st", name=f"stl_{n}_{si}"
                    )
                    emit_matmuls(n, v, st, g0, gn, 0)

                    def evl(slot, st=st, fdp=fdp):
                        return st[:, :, slot, :fdp]

                    def vall(name_, n=n, si=si, fdp=fdp):
                        tl = qpool.tile(
                            [128, 2, FD],
                            _DT,
                            tag=f"{name_}l",
                            name=f"{name_}l_{n}_{si}",
                        )
                        return tl[:, :, :fdp]

                    p = vall("p")
                    q = vall("q")
                    r = vall("r")
                    s = vall("s")
                    t = vall("t")
                    s2 = vall("s2")
                    r4 = vall("r4")
                    s8 = vall("s8")
                    u5 = vall("u5")
                    nc.vector.tensor_add(p, evl(0), evl(1))
                    nc.vector.tensor_sub(q, evl(0), evl(1))
                    nc.vector.tensor_add(r, evl(2), evl(3))
                    nc.vector.tensor_sub(s, evl(2), evl(3))
                    nc.vector.tensor_add(t, p, r)
                    nc.vector.tensor_scalar_mul(s2, s, 2.0)
                    nc.vector.tensor_scalar_mul(r4, r, 4.0)
                    nc.vector.tensor_scalar_mul(s8, s2, 4.0)
                    nc.vector.tensor_add(u5, s8, evl(5))

                    for half in range(2):
                        def as3(ap, gn=gn):
                            return ap.rearrange("p (g w) -> p g w", g=gn)

                        yt = ypool.tile(
                            [128, 28, W],
                            _DT,
                            tag="yt",
                            name=f"yl_{n}_{si}_{half}",
                        )
                        rows = 4 * gn
                        nc.vector.tensor_add(
                            yt[:, 0:rows:4, :],
                            as3(evl(4)[:, half]),
                            as3(t[:, half]),
                        )
                        nc.vector.tensor_add(
                            yt[:, 1:rows:4, :], as3(q[:, half]), as3(s2[:, half])
                        )
                        nc.vector.tensor_add(
                            yt[:, 2:rows:4, :], as3(p[:, half]), as3(r4[:, half])
                        )
                        nc.vector.tensor_add(
                            yt[:, 3:rows:4, :], as3(q[:, half]), as3(u5[:, half])
                        )

                        r0 = 4 * g0
                        y_slice = y[
                            n, half * 128 : (half + 1) * 128, r0 : r0 + rows, :
                        ]
                        if last and half == 1:
                            # split the final store so its DMA drain
                            # doesn't gate the end barrier on one queue
                            nc.sync.dma_start(
                                out=y_slice[:, 0 : 2 * gn, :],
                                in_=yt[:, 0 : 2 * gn, :],
                            )
                            nc.scalar.dma_start(
                                out=y_slice[:, 2 * gn : rows, :],
                                in_=yt[:, 2 * gn : rows, :],
                            )
                        else:
                            nc.sync.dma_start(out=y_slice, in_=yt[:, :rows, :])

            per_seg_image(0, SEGS_MID, False)

            # ---- images 1..2: whole-image epilogue (fd=784 ops amortize
            # the DVE per-op overhead; it hides under the NEXT image's
            # matmul stream).
            for n in range(1, N_PER - 1):
                v = vt[n]
                st = spool.tile(
                    [128, 2, 6, FDI], _DT, tag="sti", name=f"st_{n}"
                )
                for g0, gn in SEGS_MID:
                    emit_matmuls(n, v, st, g0, gn, g0 * W)

                # st slots (emission order): 0=m1 1=m2 2=m3 3=m4 4=m0 5=m5
                def ev(slot, st=st):
                    return st[:, :, slot, :]

                def val(name_, n=n):
                    return qpool.tile(
                        [128, 2, FDI], _DT, tag=name_, name=f"{name_}_{n}"
                    )

                p = val("p")
                q = val("q")
                r = val("r")
                s = val("s")
                t = val("t")
                s2 = val("s2")
                r4 = val("r4")
                s8 = val("s8")
                u5 = val("u5")
                nc.vector.tensor_add(p, ev(0), ev(1))
                nc.vector.tensor_sub(q, ev(0), ev(1))
                nc.vector.tensor_add(r, ev(2), ev(3))
                nc.vector.tensor_sub(s, ev(2), ev(3))
                nc.vector.tensor_add(t, p, r)
                nc.vector.tensor_scalar_mul(s2, s, 2.0)
                nc.vector.tensor_scalar_mul(r4, r, 4.0)
                nc.vector.tensor_scalar_mul(s8, s2, 4.0)
                nc.vector.tensor_add(u5, s8, ev(5))

                def emit_y(
                    halves=(0, 1), n=n, ev=ev, p=p, q=q, t=t, s2=s2, r4=r4, u5=u5
                ):
                    for half in halves:
                        def as3(ap):
                            return ap.rearrange("p (g w) -> p g w", g=G4)

                        yt = ypool.tile(
                            [128, H, W], _DT, tag="yti", name=f"y_{n}_{half}"
                        )
                        nc.vector.tensor_add(
                            yt[:, 0:H:4, :], as3(ev(4)[:, half]), as3(t[:, half])
                        )
                        nc.vector.tensor_add(
                            yt[:, 1:H:4, :], as3(q[:, half]), as3(s2[:, half])
                        )
                        nc.vector.tensor_add(
                            yt[:, 2:H:4, :], as3(p[:, half]), as3(r4[:, half])
                        )
                        nc.vector.tensor_add(
                            yt[:, 3:H:4, :], as3(q[:, half]), as3(u5[:, half])
                        )
                        nc.sync.dma_start(
                            out=y[n, half * 128 : (half + 1) * 128, :, :],
                            in_=yt[:],
                        )

                if n < N_PER - 2:
                    emit_y()
                else:
                    # defer image 2's y-writes (store-feeding only, no
                    # downstream deps) until after image 3's seg-0 vals:
                    # this releases seg-0's st tile ~4us earlier, unblocking
                    # the eviction -> PSUM-pair chain that stalls image 3's
                    # later matmuls
                    deferred_y = emit_y

            per_seg_image(N_PER - 1, SEGS_LAST[:1], False)
            deferred_y((0,))
            per_seg_image(N_PER - 1, SEGS_LAST[1:2], False, si0=1)
            deferred_y((1,))
            per_seg_image(N_PER - 1, SEGS_LAST[2:], True, si0=2)
    nc.compile()
    return nc


_NC = None


def _get_nc():
    global _NC
    if _NC is None:
        _NC = _build()
    return _NC


def _prep_inputs(x, kernels, b):
    bf16 = ml_dtypes.bfloat16
    x = np.asarray(x, dtype=np.float32)
    w = np.asarray(kernels, dtype=np.float32)  # [O, C, kh, kw]

    # host input transform V = B^T d along H (comps in emission order)
    xp = np.pad(x, ((0, 0), (0, 0), (1, 1), (0, 0)))  # rows -1..56
    # d[r] for group g is xp[:, :, 4g+r, :]
    d = [xp[:, :, r : r + 4 * G4 : 4, :] for r in range(6)]  # each [N,C,14,W]
    a = d[0] - d[2]
    i_ = d[4] - d[2]
    c = d[1] + d[2]
    e = d[3] + d[4]
    f = d[1] - d[2]
    gg = d[3] - d[4]
    h = d[3] - d[1]
    k = d[3] - d[5]
    Vfull = [4 * a + i_, -4 * c + e, 4 * f - gg, 2 * h + i_, -2 * h + i_, -4 * h - k]
    V = np.stack([Vfull[ci] for ci in _COMP_ORDER], axis=2)  # [N,C,6,14,W]
    vsb = np.ascontiguousarray(V).astype(bf16)

    # host weight transform U[i] = sum_r G[i,r] w[:,:,r,:], blocks in
    # (emission comp, kw 1/0/2) order, each [C, O]
    U = np.einsum("ir,ocrk->iock", _G, w)  # [6, O, C, kw]
    blocks = []
    for j in _COMP_ORDER:
        for kw in _KW_ORDER:
            blocks.append(U[j, :, :, kw].T)
    wtb = np.ascontiguousarray(np.concatenate(blocks, axis=1)).astype(bf16)
    return vsb, wtb


def kernel(x, kernels, b):
    nc = _get_nc()
    vsb, wtb = _prep_inputs(x, kernels, b)
    in_maps = [
        {"vs": vsb[i * N_PER : (i + 1) * N_PER], "wt": wtb}
        for i in range(N_CORES)
    ]
    res = run_bass_kernel_spmd(nc, in_maps, core_ids=list(range(N_CORES)))
    bias = np.asarray(b, dtype=np.float32).reshape(1, C_OUT, 1, 1)
    out = np.concatenate(
        [
            r["y"].astype(np.float32).reshape(N_PER, C_OUT, H, W)
            for r in res.results
        ],
        axis=0,
    )
    out += bias
    return np.ascontiguousarray(out, dtype=np.float32)
